# revision 16
# baseline (speedup 1.0000x reference)
"""Trainium2 Bass kernel for nn_DataEmbedding_Stats.

Computation: rolling-window stats (window=24, replicate-padded) over
x (B,S,7) -> 35 features -> circular conv1d(k=3) -> (B,S,512).

Strategy (8 NeuronCores, pure data parallel over batch, 4 batches/core):
 - x loaded contiguously (28B runs) into [128,128] staging tiles, then
   PE-transposed so channels land on partitions: X [128, 1047] layout,
   partition = 32j + 7b + c (j = 1024-seq chunk, b = local batch,
   c = channel), free = seq within chunk + 23-halo.
 - rolling sum/sumsq/max/min via log-doubling shifted ops on DVE
   (window 24 = combine(16-window, 8-window shifted by 16)).
 - std = sqrt(max(SQ24 - S24^2/24, 0)/23); mean folded into conv weights
   as S24 * (W_mean/24) on host.
 - per-stat contiguous ST2 [28, 4100] tiles (partition = 7b+c, col m =
   feats at seq (m-2) mod 4096) built with GPSIMD copies; F3 [106, 4098]
   per batch = 3 conv-tap shifts of the 35 features (circular) + ones
   row (bias folded as contraction row 105) via 15 wide DMAs per batch.
 - conv as matmul: per 128 positions, out[128,512] = F3[:,t+1:t+129].T
   @ Wt, float32r operands -> full PE speed at N=512.
 - PSUM -> SBUF copies split between DVE/ACT, 2MB output DMAs.
"""

import numpy as np

try:
    import concourse.bass as bass  # noqa: F401
except ImportError:
    import sys

    for _p in ("/opt/trn_rl_repo", "/root/.axon_site/_ro/trn_rl_repo"):
        if _p not in sys.path:
            sys.path.insert(0, _p)

B, S, C, W, D = 32, 4096, 7, 24, 512
NCORES = 8
BSH = B // NCORES          # batches per core
NJ = 4                     # seq chunks (row groups of 32 partitions)
CH = S // NJ               # 1024
HALO = W - 1               # 23
XCOLS = CH + HALO          # 1047
NF = 5 * C                 # 35 features
K = 3 * NF + 1             # 106 contraction rows (ones row last)
F3W = S + 2                # 4098
ST2W = S + 4               # 4100: col m = feats[(m-2) mod S]
NT = S // 128              # 32 position tiles per batch
NTR = S // 512             # 8 PE transposes (each covers 512 seq x 4 batch)
DVE_COLS = 224             # psum-copy split: DVE [0:224], ACT [224:512]
OUTG = 4                   # position tiles per output staging tile

_CACHE = {}


def _build():
    import concourse.bacc as bacc
    import concourse.tile as tile
    from concourse import mybir

    f32 = mybir.dt.float32
    bf16 = mybir.dt.bfloat16
    Alu = mybir.AluOpType
    Act = mybir.ActivationFunctionType

    nc = bacc.Bacc(
        "TRN2",
        target_bir_lowering=False,
        debug=False,
        enable_asserts=False,
        num_devices=NCORES,
    )

    x_d = nc.dram_tensor("x", (BSH, S, C), f32, kind="ExternalInput")
    wt_d = nc.dram_tensor("wt", (K, D), bf16, kind="ExternalInput")
    ones_d = nc.dram_tensor("ones", (1, F3W), bf16, kind="ExternalInput")
    id_d = nc.dram_tensor("ident", (128, 128), f32, kind="ExternalInput")
    y_d = nc.dram_tensor("y", (BSH, S, D), f32, kind="ExternalOutput")

    with tile.TileContext(nc) as tc:
        with (
            tc.tile_pool(name="stats", bufs=1) as pst,
            tc.tile_pool(name="st2p", bufs=1) as pst2,
            tc.tile_pool(name="f3p", bufs=3) as pf3,
            tc.tile_pool(name="wtp", bufs=1) as pwt,
            tc.tile_pool(name="stage_in", bufs=12) as pstg,
            tc.tile_pool(name="psT", bufs=1, space="PSUM") as psT,
            tc.tile_pool(name="psum", bufs=7, space="PSUM") as pps,
            tc.tile_pool(name="outp", bufs=6) as pout,
        ):
            wt = pwt.tile([K, D], bf16, tag="wt")
            nc.sync.dma_start(wt[:], wt_d.ap())
            ident = pwt.tile([128, 128], f32, tag="ident")
            nc.sync.dma_start(ident[:], id_d.ap())

            X = pst.tile([128, XCOLS], f32, tag="X")
            T1 = pst.tile([128, XCOLS], f32, tag="T1")
            T2 = pst.tile([128, XCOLS], f32, tag="T2")
            T3 = pst.tile([128, XCOLS], f32, tag="T3")
            T4 = pst.tile([128, XCOLS], f32, tag="T4")
            S24 = pst.tile([128, XCOLS], f32, tag="S24")
            MNT = pst.tile([128, XCOLS], f32, tag="MNT")

            # ---- load x: contiguous staging + PE transpose
            # Per 128-seq block (T, u): stg [128, 32], partition p = seq
            # offset within block, col = b*7 + c (cols 28..31 unused).
            # Transpose -> PSUM [32, 128]: partition = 7b + c, free = p.
            # Copy into X rows 32j + 7b + c (j = T//2) at the block's cols.
            _blocks = [(T, u) for T in (1, 3, 5, 7) for u in range(4)]
            _blocks += [(T, u) for T in (0, 2, 4, 6) for u in (3, 0, 1, 2)]
            for T, u in _blocks:
                j = T // 2
                c0 = HALO + 512 * (T % 2)
                if True:
                    s0 = 512 * T + 128 * u
                    stg = pstg.tile([128, 32], f32, tag="stg")
                    eng = (nc.sync, nc.scalar, nc.gpsimd)[(T * 4 + u) % 3]
                    eng.dma_start(
                        stg[:, 0:28],
                        x_d.ap()[:, s0 : s0 + 128, :].rearrange("b p c -> p b c"),
                    )
                    pst_t = psT.tile([32, 128], f32, tag="pst_t")
                    nc.tensor.transpose(pst_t[0:28, :], stg[:, 0:28], ident[:])
                    nc.scalar.copy(
                        X[32 * j : 32 * j + 28, c0 + 128 * u : c0 + 128 * (u + 1)],
                        pst_t[0:28, :],
                    )
                    if T % 2 == 1 and u == 3 and j + 1 < NJ:
                        # back-halo for chunk j+1: seq 1024(j+1)-23 .. -1
                        nc.scalar.copy(
                            X[32 * (j + 1) : 32 * (j + 1) + 28, 0:HALO],
                            pst_t[0:28, 128 - HALO : 128],
                        )
            # ---- per-stat contiguous ST2 [28, 4100] (GPSIMD copies)
            # ST2_t[7b+c, m] = feats_t[b, c, (m-2) mod 4096]
            ST2 = [
                pst2.tile([28, ST2W], bf16, tag=f"ST2_{t}", name=f"ST2_{t}")
                for t in range(5)
            ]

            def relayout(t, st):
                st2 = ST2[t]
                for j in range(NJ):
                    nc.scalar.copy(
                        st2[:, 2 + CH * j : 2 + CH * (j + 1)],
                        st[32 * j : 32 * j + 28, HALO : HALO + CH],
                    )
                # wrap cols: 0:2 <- seq 4094..4095 ; 4098:4100 <- seq 0..1
                nc.scalar.copy(st2[:, 0:2], st[96:124, XCOLS - 2 : XCOLS])
                nc.scalar.copy(st2[:, S + 2 : S + 4], st[0:28, HALO : HALO + 2])

            relayout(0, X)  # raw x (ready right after the loader)

            # ---- rolling stats (all [128, *], shifts along free dim)
            E = XCOLS  # 1047

            def tt(dst, d0, a, a0, bsrc, b0, n, op):
                nc.vector.tensor_tensor(
                    dst[:, d0 : d0 + n], a[:, a0 : a0 + n], bsrc[:, b0 : b0 + n], op
                )

            # Two-wave column-split stats: R-wave (dest cols >= 513..535
            # ladder) only needs X cols >= 407 (odd seq-blocks + even u=3);
            # L-wave needs the rest incl. halos. Levels shift by 1/2/4/8/16.
            def stats_wave(s1, s2, s3, s4, s5, e, sx0):
                # sum chain: A(T1) B(T2) C(T3) D(T1) S24
                tt(T1, s1, X, s1, X, s1 - 1, e - s1, Alu.add)
                tt(T2, s2, T1, s2, T1, s2 - 2, e - s2, Alu.add)
                tt(T3, s3, T2, s3, T2, s3 - 4, e - s3, Alu.add)
                tt(T1, s4, T3, s4, T3, s4 - 8, e - s4, Alu.add)
                tt(S24, s5, T1, s5, T3, s5 - 16, e - s5, Alu.add)
                # squares: SQX(T4, ACT) A2(T1) B2(T2) C2(T3) D2(T1) SQ24(T2)
                nc.scalar.square(T4[:, sx0:e], X[:, sx0:e])
                tt(T1, s1, T4, s1, T4, s1 - 1, e - s1, Alu.add)
                tt(T2, s2, T1, s2, T1, s2 - 2, e - s2, Alu.add)
                tt(T3, s3, T2, s3, T2, s3 - 4, e - s3, Alu.add)
                tt(T1, s4, T3, s4, T3, s4 - 8, e - s4, Alu.add)
                tt(T2, s5, T1, s5, T3, s5 - 16, e - s5, Alu.add)
                # var/std: M24sq(T4) VARraw(T3) VARcl(T2) SD(T4)
                nc.scalar.activation(
                    T4[:, s5:e], S24[:, s5:e], Act.Square, 0.0, float(W**-0.5)
                )
                tt(T3, s5, T2, s5, T4, s5, e - s5, Alu.subtract)
                nc.vector.tensor_scalar(T2[:, s5:e], T3[:, s5:e], 0.0, None, Alu.max)
                nc.scalar.activation(
                    T4[:, s5:e], T2[:, s5:e], Act.Sqrt, 0.0, 1.0 / (W - 1)
                )
                # max chain: MA(T1) MB(T3) MC(T1) MD(T3) MX(T2)
                tt(T1, s1, X, s1, X, s1 - 1, e - s1, Alu.max)
                tt(T3, s2, T1, s2, T1, s2 - 2, e - s2, Alu.max)
                tt(T1, s3, T3, s3, T3, s3 - 4, e - s3, Alu.max)
                tt(T3, s4, T1, s4, T1, s4 - 8, e - s4, Alu.max)
                tt(T2, s5, T3, s5, T1, s5 - 16, e - s5, Alu.max)
                # min chain: NA(T1) NB(T3) NC(T1) ND(T3) MN(MNT)
                tt(T1, s1, X, s1, X, s1 - 1, e - s1, Alu.min)
                tt(T3, s2, T1, s2, T1, s2 - 2, e - s2, Alu.min)
                tt(T1, s3, T3, s3, T3, s3 - 4, e - s3, Alu.min)
                tt(T3, s4, T1, s4, T1, s4 - 8, e - s4, Alu.min)
                tt(MNT, s5, T3, s5, T1, s5 - 16, e - s5, Alu.min)

            stats_wave(513, 515, 519, 527, 535, E, 512)   # R-wave
            # j=0 halo: replicate x[b,0,c] into cols 0..22 (between waves:
            # only the L-wave needs it; keeps DVE from stalling early)
            nc.vector.tensor_scalar(
                X[0:28, 0:HALO],
                X[0:28, HALO : 2 * HALO],
                0.0,
                X[0:28, HALO : HALO + 1],
                Alu.mult,
                Alu.add,
            )
            stats_wave(1, 3, 7, 15, 23, 535, 0)           # L-wave
            relayout(1, S24)  # mean (raw window sum; /24 folded into weights)
            relayout(4, T4)   # std
            relayout(2, T2)   # max
            relayout(3, MNT)  # min

            # ---- per batch: build F3 (block k = ST2[:, k:k+4098]) + matmuls
            def build_f3(b):
                f3 = pf3.tile([K, F3W], bf16, tag="F3", name=f"f3_{b}")
                if b < 3:
                    # bufs=3 slot rotation reuses the same SBUF; the ones row
                    # is identical every round, so write each slot just once.
                    nc.gpsimd.dma_start(f3[K - 1 : K, :], ones_d.ap())
                for t in range(5):
                    for k in range(3):
                        r0 = 35 * k + 7 * t
                        nc.gpsimd.dma_start(
                            f3[r0 : r0 + 7, :],
                            ST2[t][7 * b : 7 * b + 7, k : k + F3W],
                        )
                return f3

            f3_cur = build_f3(0)
            for b in range(BSH):
                f3_next = build_f3(b + 1) if b + 1 < BSH else None
                for g in range(NT // OUTG):
                    stage = pout.tile([128, OUTG * D], f32, tag="stage")
                    for q in range(OUTG):
                        t0 = 128 * (OUTG * g + q)
                        ps = pps.tile([128, D], f32, tag="ps")
                        nc.tensor.matmul(
                            ps[:],
                            f3_cur[:, t0 + 1 : t0 + 129],
                            wt[:],
                            start=True,
                            stop=True,
                        )
                        c0 = D * q
                        ceng = nc.vector.tensor_copy if q % 2 == 0 else nc.scalar.copy
                        ceng(stage[:, c0 : c0 + D], ps[:])
                    deng = nc.sync if (b * 4 + g) % 2 == 0 else nc.scalar
                    deng.dma_start(
                        y_d.ap()[
                            b, 128 * OUTG * g : 128 * OUTG * (g + 1), :
                        ].rearrange("(q p) d -> p q d", p=128),
                        stage[:].rearrange("p (q d) -> p q d", q=OUTG),
                    )
                f3_cur = f3_next

    nc.compile()
    return nc


def _prep_host(W_conv, b_conv):
    import ml_dtypes

    wt = np.empty((K, D), np.float32)
    wkf = np.ascontiguousarray(W_conv.transpose(2, 1, 0)).copy()  # (3, 35, 512)
    wkf[:, C : 2 * C, :] *= 1.0 / W  # fold mean = S24/24 into weights
    wt[: K - 1] = wkf.reshape(3 * NF, D)
    wt[K - 1] = b_conv.astype(np.float32)
    return wt.astype(ml_dtypes.bfloat16)


def _run(x, W_conv, b_conv, trace=False, **kw):
    from concourse import bass_utils

    if "nc" not in _CACHE:
        _CACHE["nc"] = _build()
    nc = _CACHE["nc"]

    wt = _prep_host(np.asarray(W_conv), np.asarray(b_conv))
    import ml_dtypes

    ones = np.ones((1, F3W), ml_dtypes.bfloat16)
    ident = np.eye(128, dtype=np.float32)
    x = np.ascontiguousarray(np.asarray(x, np.float32))
    in_maps = [
        {"x": x[BSH * i : BSH * (i + 1)], "wt": wt, "ones": ones, "ident": ident}
        for i in range(NCORES)
    ]
    res = bass_utils.run_bass_kernel_spmd(
        nc, in_maps, core_ids=list(range(NCORES)), trace=trace, **kw
    )
    out = np.concatenate([r["y"] for r in res.results], axis=0)
    return out, res


def kernel(x, x_mark=None, W_conv=None, b_conv=None, **_unused):
    out, _ = _run(x, W_conv, b_conv, trace=False)
    return out


# revision 17
# speedup vs baseline: 1.0317x; 1.0317x over previous
"""Trainium2 Bass kernel for nn_DataEmbedding_Stats.

Computation: rolling-window stats (window=24, replicate-padded) over
x (B,S,7) -> 35 features -> circular conv1d(k=3) -> (B,S,512).

Strategy (8 NeuronCores, pure data parallel over batch, 4 batches/core):
 - x loaded contiguously (28B runs) into [128,128] staging tiles, then
   PE-transposed so channels land on partitions: X [128, 1047] layout,
   partition = 32j + 7b + c (j = 1024-seq chunk, b = local batch,
   c = channel), free = seq within chunk + 23-halo.
 - rolling sum/sumsq/max/min via log-doubling shifted ops on DVE
   (window 24 = combine(16-window, 8-window shifted by 16)).
 - std = sqrt(max(SQ24 - S24^2/24, 0)/23); mean folded into conv weights
   as S24 * (W_mean/24) on host.
 - per-stat contiguous ST2 [28, 4100] tiles (partition = 7b+c, col m =
   feats at seq (m-2) mod 4096) built with GPSIMD copies; F3 [106, 4098]
   per batch = 3 conv-tap shifts of the 35 features (circular) + ones
   row (bias folded as contraction row 105) via 15 wide DMAs per batch.
 - conv as matmul: per 128 positions, out[128,512] = F3[:,t+1:t+129].T
   @ Wt, float32r operands -> full PE speed at N=512.
 - PSUM -> SBUF copies split between DVE/ACT, 2MB output DMAs.
"""

import numpy as np

try:
    import concourse.bass as bass  # noqa: F401
except ImportError:
    import sys

    for _p in ("/opt/trn_rl_repo", "/root/.axon_site/_ro/trn_rl_repo"):
        if _p not in sys.path:
            sys.path.insert(0, _p)

B, S, C, W, D = 32, 4096, 7, 24, 512
NCORES = 8
BSH = B // NCORES          # batches per core
NJ = 4                     # seq chunks (row groups of 32 partitions)
CH = S // NJ               # 1024
HALO = W - 1               # 23
XCOLS = CH + HALO          # 1047
NF = 5 * C                 # 35 features
K = 3 * NF + 1             # 106 contraction rows (ones row last)
F3W = S + 2                # 4098
ST2W = S + 4               # 4100: col m = feats[(m-2) mod S]
NT = S // 128              # 32 position tiles per batch
NTR = S // 512             # 8 PE transposes (each covers 512 seq x 4 batch)
DVE_COLS = 224             # psum-copy split: DVE [0:224], ACT [224:512]
OUTG = 4                   # position tiles per output staging tile

_CACHE = {}


def _build():
    import concourse.bacc as bacc
    import concourse.tile as tile
    from concourse import mybir

    f32 = mybir.dt.float32
    bf16 = mybir.dt.bfloat16
    Alu = mybir.AluOpType
    Act = mybir.ActivationFunctionType

    nc = bacc.Bacc(
        "TRN2",
        target_bir_lowering=False,
        debug=False,
        enable_asserts=False,
        num_devices=NCORES,
    )

    x_d = nc.dram_tensor("x", (BSH, S, C), f32, kind="ExternalInput")
    wt_d = nc.dram_tensor("wt", (K, D), bf16, kind="ExternalInput")
    ones_d = nc.dram_tensor("ones", (1, F3W), bf16, kind="ExternalInput")
    id_d = nc.dram_tensor("ident", (128, 128), f32, kind="ExternalInput")
    y_d = nc.dram_tensor("y", (BSH, S, D), f32, kind="ExternalOutput")

    with tile.TileContext(nc) as tc:
        with (
            tc.tile_pool(name="stats", bufs=1) as pst,
            tc.tile_pool(name="st2p", bufs=1) as pst2,
            tc.tile_pool(name="f3p", bufs=3) as pf3,
            tc.tile_pool(name="wtp", bufs=1) as pwt,
            tc.tile_pool(name="stage_in", bufs=12) as pstg,
            tc.tile_pool(name="psT", bufs=2, space="PSUM") as psT,
            tc.tile_pool(name="psum", bufs=6, space="PSUM") as pps,
            tc.tile_pool(name="outp", bufs=6) as pout,
        ):
            wt = pwt.tile([K, D], bf16, tag="wt")
            nc.sync.dma_start(wt[:], wt_d.ap())
            ident = pwt.tile([128, 128], f32, tag="ident")
            nc.sync.dma_start(ident[:], id_d.ap())

            X = pst.tile([128, XCOLS], f32, tag="X")
            T1 = pst.tile([128, XCOLS], f32, tag="T1")
            T2 = pst.tile([128, XCOLS], f32, tag="T2")
            T3 = pst.tile([128, XCOLS], f32, tag="T3")
            T4 = pst.tile([128, XCOLS], f32, tag="T4")
            S24 = pst.tile([128, XCOLS], f32, tag="S24")
            MNT = pst.tile([128, XCOLS], f32, tag="MNT")

            # ---- load x: contiguous staging + PE transpose
            # Per 128-seq block (T, u): stg [128, 32], partition p = seq
            # offset within block, col = b*7 + c (cols 28..31 unused).
            # Transpose -> PSUM [32, 128]: partition = 7b + c, free = p.
            # Copy into X rows 32j + 7b + c (j = T//2) at the block's cols.
            _blocks = [(T, u) for T in (1, 3, 5, 7) for u in range(4)]
            _blocks += [(T, u) for T in (0, 2, 4, 6) for u in (3, 0, 1, 2)]
            for T, u in _blocks:
                j = T // 2
                c0 = HALO + 512 * (T % 2)
                if True:
                    s0 = 512 * T + 128 * u
                    stg = pstg.tile([128, 32], f32, tag="stg")
                    eng = (nc.sync, nc.scalar, nc.gpsimd)[(T * 4 + u) % 3]
                    eng.dma_start(
                        stg[:, 0:28],
                        x_d.ap()[:, s0 : s0 + 128, :].rearrange("b p c -> p b c"),
                    )
                    pst_t = psT.tile([32, 128], f32, tag="pst_t")
                    nc.tensor.transpose(pst_t[0:28, :], stg[:, 0:28], ident[:])
                    nc.scalar.copy(
                        X[32 * j : 32 * j + 28, c0 + 128 * u : c0 + 128 * (u + 1)],
                        pst_t[0:28, :],
                    )
                    if T % 2 == 1 and u == 3 and j + 1 < NJ:
                        # back-halo for chunk j+1: seq 1024(j+1)-23 .. -1
                        nc.scalar.copy(
                            X[32 * (j + 1) : 32 * (j + 1) + 28, 0:HALO],
                            pst_t[0:28, 128 - HALO : 128],
                        )
            # ---- per-stat contiguous ST2 [28, 4100] (GPSIMD copies)
            # ST2_t[7b+c, m] = feats_t[b, c, (m-2) mod 4096]
            ST2 = [
                pst2.tile([28, ST2W], bf16, tag=f"ST2_{t}", name=f"ST2_{t}")
                for t in range(5)
            ]

            def relayout(t, st):
                st2 = ST2[t]
                for j in range(NJ):
                    nc.scalar.copy(
                        st2[:, 2 + CH * j : 2 + CH * (j + 1)],
                        st[32 * j : 32 * j + 28, HALO : HALO + CH],
                    )
                # wrap cols: 0:2 <- seq 4094..4095 ; 4098:4100 <- seq 0..1
                nc.scalar.copy(st2[:, 0:2], st[96:124, XCOLS - 2 : XCOLS])
                nc.scalar.copy(st2[:, S + 2 : S + 4], st[0:28, HALO : HALO + 2])

            relayout(0, X)  # raw x (ready right after the loader)

            # ---- rolling stats (all [128, *], shifts along free dim)
            E = XCOLS  # 1047

            def tt(dst, d0, a, a0, bsrc, b0, n, op):
                nc.vector.tensor_tensor(
                    dst[:, d0 : d0 + n], a[:, a0 : a0 + n], bsrc[:, b0 : b0 + n], op
                )

            # Two-wave column-split stats: R-wave (dest cols >= 513..535
            # ladder) only needs X cols >= 407 (odd seq-blocks + even u=3);
            # L-wave needs the rest incl. halos. Levels shift by 1/2/4/8/16.
            def stats_wave(s1, s2, s3, s4, s5, e, sx0):
                # sum chain: A(T1) B(T2) C(T3) D(T1) S24
                tt(T1, s1, X, s1, X, s1 - 1, e - s1, Alu.add)
                tt(T2, s2, T1, s2, T1, s2 - 2, e - s2, Alu.add)
                tt(T3, s3, T2, s3, T2, s3 - 4, e - s3, Alu.add)
                tt(T1, s4, T3, s4, T3, s4 - 8, e - s4, Alu.add)
                tt(S24, s5, T1, s5, T3, s5 - 16, e - s5, Alu.add)
                # squares: SQX(T4, ACT) A2(T1) B2(T2) C2(T3) D2(T1) SQ24(T2)
                nc.scalar.square(T4[:, sx0:e], X[:, sx0:e])
                tt(T1, s1, T4, s1, T4, s1 - 1, e - s1, Alu.add)
                tt(T2, s2, T1, s2, T1, s2 - 2, e - s2, Alu.add)
                tt(T3, s3, T2, s3, T2, s3 - 4, e - s3, Alu.add)
                tt(T1, s4, T3, s4, T3, s4 - 8, e - s4, Alu.add)
                tt(T2, s5, T1, s5, T3, s5 - 16, e - s5, Alu.add)
                # var/std: M24sq(T4) VARraw(T3) VARcl(T2) SD(T4)
                nc.scalar.activation(
                    T4[:, s5:e], S24[:, s5:e], Act.Square, 0.0, float(W**-0.5)
                )
                tt(T3, s5, T2, s5, T4, s5, e - s5, Alu.subtract)
                nc.vector.tensor_scalar(T2[:, s5:e], T3[:, s5:e], 0.0, None, Alu.max)
                nc.scalar.activation(
                    T4[:, s5:e], T2[:, s5:e], Act.Sqrt, 0.0, 1.0 / (W - 1)
                )
                # max chain: MA(T1) MB(T3) MC(T1) MD(T3) MX(T2)
                tt(T1, s1, X, s1, X, s1 - 1, e - s1, Alu.max)
                tt(T3, s2, T1, s2, T1, s2 - 2, e - s2, Alu.max)
                tt(T1, s3, T3, s3, T3, s3 - 4, e - s3, Alu.max)
                tt(T3, s4, T1, s4, T1, s4 - 8, e - s4, Alu.max)
                tt(T2, s5, T3, s5, T1, s5 - 16, e - s5, Alu.max)
                # min chain: NA(T1) NB(T3) NC(T1) ND(T3) MN(MNT)
                tt(T1, s1, X, s1, X, s1 - 1, e - s1, Alu.min)
                tt(T3, s2, T1, s2, T1, s2 - 2, e - s2, Alu.min)
                tt(T1, s3, T3, s3, T3, s3 - 4, e - s3, Alu.min)
                tt(T3, s4, T1, s4, T1, s4 - 8, e - s4, Alu.min)
                tt(MNT, s5, T3, s5, T1, s5 - 16, e - s5, Alu.min)

            stats_wave(513, 515, 519, 527, 535, E, 512)   # R-wave
            # j=0 halo: replicate x[b,0,c] into cols 0..22 (between waves:
            # only the L-wave needs it; keeps DVE from stalling early)
            nc.vector.tensor_scalar(
                X[0:28, 0:HALO],
                X[0:28, HALO : 2 * HALO],
                0.0,
                X[0:28, HALO : HALO + 1],
                Alu.mult,
                Alu.add,
            )
            stats_wave(1, 3, 7, 15, 23, 535, 0)           # L-wave
            relayout(1, S24)  # mean (raw window sum; /24 folded into weights)
            relayout(4, T4)   # std
            relayout(2, T2)   # max
            relayout(3, MNT)  # min

            # ---- per batch: build F3 (block k = ST2[:, k:k+4098]) + matmuls
            def build_f3(b):
                f3 = pf3.tile([K, F3W], bf16, tag="F3", name=f"f3_{b}")
                if b < 3:
                    # bufs=3 slot rotation reuses the same SBUF; the ones row
                    # is identical every round, so write each slot just once.
                    nc.gpsimd.dma_start(f3[K - 1 : K, :], ones_d.ap())
                for t in range(5):
                    for k in range(3):
                        r0 = 35 * k + 7 * t
                        nc.gpsimd.dma_start(
                            f3[r0 : r0 + 7, :],
                            ST2[t][7 * b : 7 * b + 7, k : k + F3W],
                        )
                return f3

            f3_cur = build_f3(0)
            for b in range(BSH):
                f3_next = build_f3(b + 1) if b + 1 < BSH else None
                for g in range(NT // OUTG):
                    stage = pout.tile([128, OUTG * D], f32, tag="stage")
                    for q in range(OUTG):
                        t0 = 128 * (OUTG * g + q)
                        ps = pps.tile([128, D], f32, tag="ps")
                        nc.tensor.matmul(
                            ps[:],
                            f3_cur[:, t0 + 1 : t0 + 129],
                            wt[:],
                            start=True,
                            stop=True,
                        )
                        c0 = D * q
                        ceng = nc.vector.tensor_copy if q % 2 == 0 else nc.scalar.copy
                        ceng(stage[:, c0 : c0 + D], ps[:])
                    deng = nc.sync if (b * 4 + g) % 2 == 0 else nc.scalar
                    deng.dma_start(
                        y_d.ap()[
                            b, 128 * OUTG * g : 128 * OUTG * (g + 1), :
                        ].rearrange("(q p) d -> p q d", p=128),
                        stage[:].rearrange("p (q d) -> p q d", q=OUTG),
                    )
                f3_cur = f3_next

    nc.compile()
    return nc


def _prep_host(W_conv, b_conv):
    import ml_dtypes

    wt = np.empty((K, D), np.float32)
    wkf = np.ascontiguousarray(W_conv.transpose(2, 1, 0)).copy()  # (3, 35, 512)
    wkf[:, C : 2 * C, :] *= 1.0 / W  # fold mean = S24/24 into weights
    wt[: K - 1] = wkf.reshape(3 * NF, D)
    wt[K - 1] = b_conv.astype(np.float32)
    return wt.astype(ml_dtypes.bfloat16)


def _run(x, W_conv, b_conv, trace=False, **kw):
    from concourse import bass_utils

    if "nc" not in _CACHE:
        _CACHE["nc"] = _build()
    nc = _CACHE["nc"]

    wt = _prep_host(np.asarray(W_conv), np.asarray(b_conv))
    import ml_dtypes

    ones = np.ones((1, F3W), ml_dtypes.bfloat16)
    ident = np.eye(128, dtype=np.float32)
    x = np.ascontiguousarray(np.asarray(x, np.float32))
    in_maps = [
        {"x": x[BSH * i : BSH * (i + 1)], "wt": wt, "ones": ones, "ident": ident}
        for i in range(NCORES)
    ]
    res = bass_utils.run_bass_kernel_spmd(
        nc, in_maps, core_ids=list(range(NCORES)), trace=trace, **kw
    )
    out = np.concatenate([r["y"] for r in res.results], axis=0)
    return out, res


def kernel(x, x_mark=None, W_conv=None, b_conv=None, **_unused):
    out, _ = _run(x, W_conv, b_conv, trace=False)
    return out
